# revision 39
# baseline (speedup 1.0000x reference)
"""Trainium2 Bass kernel for nn_BaseAtt (attention pooling).

reference:
    target = target_feats @ W.T                      # [B, 128]
    alpha  = softmax(mask(nf @ target), axis=k)      # [B, 200]
    onf    = sum_k alpha * nf                        # [B, 128]
    onl    = sum_k alpha * nl                        # [B, 128]

Sharding: data-parallel over B across 8 cores (512 batches/core).

Per-core pipeline (block = 16 batches):
  - target.T [128d, 512b] via 8 accumulated f16 matmuls (host-transposed
    W.T and target_feats.T inputs).
  - alpha rows: per-batch matmul with zero-masked stationary columns
    (z[:, i, :] = targetT col i on diag col i, else 0) against host-
    transposed f16 nfT, accumulating into one PSUM tile [16, 200] so all
    16 alpha rows land on partitions 0..15.
  - standard softmax in b-partition layout.
  - weighted sums: same masked-stationary trick with alpha.T columns
    against the k-partition nf|nl tiles, accumulating [16, 256] rows.
All bulk tensors ride HBM as f16 (the 2e-2 tolerance leaves ~3x margin);
fp8 was tested numerically and fails (softmax is peaked, so quantization
of nf|nl transfers ~1:1 into the output).
Blocks are software-pipelined (alpha of block bb issues before weighted
of bb-1) so the PE never idles across the softmax dependency.  Big DMA
loads are spread over the sync/scalar HWDGE rings and the gpsimd SWDGE
ring.
"""

import numpy as np

B, K, D, FD = 4096, 200, 128, 1024
NCORES = 8
BC = B // NCORES          # 512 batches per core
BLK = 16                  # softmax / MM-accumulation / DMA block
NG = BC // BLK            # 32 groups
K0, K1 = 128, K - 128     # k-chunk sizes (128 + 72)
NEG = -60000.0            # mask fill for logits (f16-safe; exp -> 0)


def gen_kernel():
    import concourse.bacc as bacc
    import concourse.tile as tile
    from concourse import mybir

    f32 = mybir.dt.float32
    f16 = mybir.dt.float16
    AX = mybir.AxisListType
    AF = mybir.ActivationFunctionType

    nc = bacc.Bacc()

    tftr = nc.declare_dram_parameter("tftr", [128, 8, BC], f16, isOutput=False)
    wtr = nc.declare_dram_parameter("wtr", [128, 8, D], f16, isOutput=False)
    xh = nc.declare_dram_parameter("xh", [K, BC, 2 * D], f16, isOutput=False)
    nfth = nc.declare_dram_parameter("nfth", [D, BC, K], f16, isOutput=False)
    lmask = nc.declare_dram_parameter("lmask", [BLK, NG, K], f16, isOutput=False)
    ident = nc.declare_dram_parameter("ident", [128, 128], f16, isOutput=False)
    m32h = nc.declare_dram_parameter("m32h", [128, BLK, BLK], f16, isOutput=False)

    out2 = nc.declare_dram_parameter("out2", [BC, 2 * D], f32, isOutput=True)

    with tile.TileContext(nc) as tc:
        with (
            tc.tile_pool(name="const", bufs=1) as const,
            tc.tile_pool(name="xin", bufs=3) as xin,
            tc.tile_pool(name="nftile", bufs=3) as nftp,
            tc.tile_pool(name="sm", bufs=2) as sm,
            tc.tile_pool(name="zp", bufs=2) as zp,
            tc.tile_pool(name="outp", bufs=4) as outp,
            tc.tile_pool(name="pst", bufs=1, space="PSUM") as pst,
            tc.tile_pool(name="psa", bufs=2, space="PSUM") as psa,
            tc.tile_pool(name="psoa", bufs=2, space="PSUM") as psoa,
            tc.tile_pool(name="psob", bufs=2, space="PSUM") as psob,
            tc.tile_pool(name="psx", bufs=1, space="PSUM") as psx,
        ):
            # ---- setup: constants ----
            id_t = const.tile([128, 128], f16)
            nc.gpsimd.dma_start(out=id_t, in_=ident[:, :])
            m32h_t = const.tile([128, BLK, BLK], f16)
            nc.gpsimd.dma_start(out=m32h_t, in_=m32h[:, :, :])
            # lmask tile is loaded once, after the first pair's x0 (issued
            # in the main-loop section so the sync ring serves wt/tft first)
            lm_all = const.tile([BLK, NG, K], f16)
            with tc.tile_pool(name="setup", bufs=1) as setup:
                # host-prepacked layouts: large contiguous per-partition
                # runs, loaded FIRST on the sync ring so they can't starve
                # behind the prefetch flood (cross-queue round-robin offers
                # no ordering; FIFO within one queue does)
                wt_t = setup.tile([128, 8, D], f16)
                nc.sync.dma_start(out=wt_t, in_=wtr[:, :, :])
                tft_t = setup.tile([128, 8, BC], f16)
                nc.sync.dma_start(out=tft_t, in_=tftr[:, :, :])

                # ---- target.T = W @ tf.T : [128 d, BC b] ----
                ps_t = pst.tile([128, BC], f32)
                for fb in range(8):
                    nc.tensor.matmul(
                        ps_t, wt_t[:, fb, :], tft_t[:, fb, :],
                        start=(fb == 0), stop=(fb == 7),
                    )
                targetT = const.tile([128, BC], f16)
                nc.vector.tensor_copy(out=targetT, in_=ps_t)

            xtiles = {}

            def load_x2(bb):
                """Pair load: groups bb, bb+1 in one DMA per stream.

                Doubles the per-partition contiguous run (16-26 KB) so the
                SDMA descriptor fixed costs amortize better.  With bufs=4
                on the x0/nfT pools the scalar-ring issue never waits on a
                pool slot, so it cannot stall the ACT queue (softmax exp).
                """
                b0 = bb * BLK
                x0 = xin.tile([128, 2 * BLK, 256], f16, tag="x0p", bufs=3)
                x1 = xin.tile([K1, 2 * BLK, 256], f16, tag="x1p", bufs=3)
                nft_t = nftp.tile([128, 2 * BLK * K], f16, tag="nftp", bufs=4)
                nc.sync.dma_start(out=x0, in_=xh[0:K0, b0 : b0 + 2 * BLK, :])
                nc.gpsimd.dma_start(out=x1, in_=xh[K0:K, b0 : b0 + 2 * BLK, :])
                nc.scalar.dma_start(
                    out=nft_t,
                    in_=nfth[:, b0 : b0 + 2 * BLK, :].rearrange(
                        "d b k -> d (b k)"
                    ),
                )
                for j in range(2):
                    xtiles[bb + j] = (x0, x1, nft_t, j)

            def build_z(bb):
                b0 = bb * BLK
                z_t = zp.tile([128, BLK, BLK], f16, tag="z")
                nc.vector.tensor_mul(
                    out=z_t,
                    in0=targetT[:, b0 : b0 + BLK].unsqueeze(2).broadcast_to(
                        [128, BLK, BLK]
                    ),
                    in1=m32h_t,
                )
                return z_t

            def alpha_phase(bb, z_t, sm_prev):
                """Per-batch alpha-row matmuls against host-transposed nfT.

                Injects the previous block's alpha.T/za prep into the middle
                of this block's PE stream so za is ready (built on DVE in the
                shadow of these matmuls) by the time weighted_phase issues.
                """
                ps_a = psa.tile([BLK, K], f32, tag="psa")
                x0, x1, nft_t, goff = xtiles.pop(bb)
                w_ready = None
                for i in range(BLK):
                    if i == 2 and sm_prev is not None:
                        w_ready = prep_weighted(sm_prev)
                    # alpha row i accumulates into ps_a (host-transposed nfT)
                    gi = goff * BLK + i
                    nc.tensor.matmul(
                        ps_a, z_t[:, i, :], nft_t[:, gi * K : (gi + 1) * K],
                        start=(i == 0), stop=(i == BLK - 1),
                    )
                if sm_prev is not None and w_ready is None:
                    w_ready = prep_weighted(sm_prev)
                return bb, ps_a, x0, x1, goff, w_ready

            def softmax_phase(state):
                """Softmax arithmetic on DVE/ACT only (no PE instructions)."""
                bb, ps_a, x0, x1, goff = state
                aM = sm.tile([BLK, K], f32, tag="am")
                nc.vector.tensor_add(out=aM, in0=ps_a, in1=lm_all[:, bb, :])
                mx = sm.tile([BLK, 1], f32, tag="mx")
                nc.vector.reduce_max(out=mx, in_=aM, axis=AX.X)
                negmx = sm.tile([BLK, 1], f32, tag="negmx")
                nc.vector.tensor_scalar_mul(out=negmx, in0=mx, scalar1=-1.0)
                aE = sm.tile([BLK, K], f32, tag="ae")
                s_t = sm.tile([BLK, 1], f32, tag="s")
                nc.scalar.activation(
                    out=aE, in_=aM, func=AF.Exp, bias=negmx, scale=1.0,
                    accum_out=s_t,
                )
                rs = sm.tile([BLK, 1], f32, tag="rs")
                nc.vector.reciprocal(out=rs, in_=s_t)
                aN = sm.tile([BLK, K], f16, tag="an")
                nc.vector.tensor_scalar_mul(out=aN, in0=aE, scalar1=rs)
                return bb, aN, x0, x1, goff

            def prep_weighted(smstate):
                """alpha.T PE transposes + za stationary builds."""
                bb, aN, x0, x1, goff = smstate
                # alpha.T via PE transpose: [200 k, 16 b]
                ps_aT = psx.tile([128, 2 * BLK], f16, tag="pat")
                nc.tensor.transpose(ps_aT[:, 0:BLK], aN[:, 0:K0], id_t[:BLK, :BLK])
                nc.tensor.transpose(
                    ps_aT[:K1, BLK : 2 * BLK], aN[:, K0:K], id_t[:BLK, :BLK]
                )
                aT0 = zp.tile([128, BLK], f16, tag="at0")
                nc.scalar.copy(out=aT0, in_=ps_aT[:, 0:BLK])
                aT1 = zp.tile([K1, BLK], f16, tag="at1")
                nc.vector.tensor_copy(out=aT1, in_=ps_aT[:K1, BLK : 2 * BLK])

                # za stationaries, split into batch halves: the A half
                # (batches 0-7) uses diag cols 0-7 -> PSUM partitions 0-7;
                # the B half (batches 8-15) also uses diag cols 0-7 so its
                # rows land 32-aligned at partitions 32-39 under col-group 1.
                hb = BLK // 2
                m8 = m32h_t[:, 0:hb, 0:hb]
                zaA0 = zp.tile([128, hb, hb], f16, tag="zaA0")
                nc.vector.tensor_mul(
                    out=zaA0,
                    in0=aT0[:, 0:hb].unsqueeze(2).broadcast_to([128, hb, hb]),
                    in1=m8,
                )
                zaB0 = zp.tile([128, hb, hb], f16, tag="zaB0")
                nc.vector.tensor_mul(
                    out=zaB0,
                    in0=aT0[:, hb:BLK].unsqueeze(2).broadcast_to([128, hb, hb]),
                    in1=m8,
                )
                zaA1 = zp.tile([K1, hb, hb], f16, tag="zaA1")
                nc.vector.tensor_mul(
                    out=zaA1,
                    in0=aT1[:, 0:hb].unsqueeze(2).broadcast_to([K1, hb, hb]),
                    in1=m8[:K1],
                )
                zaB1 = zp.tile([K1, hb, hb], f16, tag="zaB1")
                nc.vector.tensor_mul(
                    out=zaB1,
                    in0=aT1[:, hb:BLK].unsqueeze(2).broadcast_to([K1, hb, hb]),
                    in1=m8[:K1],
                )
                return bb, x0, x1, goff, (zaA0, zaB0, zaA1, zaB1)

            def weighted_phase(wstate):
                """Weighted-sum matmuls + output copy/DMA.

                Column-tiled: batches 0-7 accumulate in PSUM bank A at
                partitions 0-15 (PE col group 0) while batches 8-15
                accumulate in bank B at partitions 32-47 (col group 1) —
                the two chains run concurrently on disjoint PE column
                groups, halving the weighted-phase span.
                """
                bb, x0, x1, goff, (zaA0, zaB0, zaA1, zaB1) = wstate
                b0 = bb * BLK
                hb = BLK // 2
                ps_a_w = psoa.tile([hb, 256], f32, tag="psoa")
                ps_b_w = psob.tile([32 + hb, 256], f32, tag="psob")
                pB = ps_b_w[32 : 32 + hb, :]
                for i in range(hb):
                    gi = goff * BLK + i
                    gj = gi + hb
                    nc.tensor.matmul(
                        ps_a_w, zaA0[:, i, :], x0[:, gi, :],
                        start=(i == 0), stop=False,
                    )
                    nc.tensor.matmul(
                        pB, zaB0[:, i, :], x0[:, gj, :],
                        start=(i == 0), stop=False,
                    )
                    nc.tensor.matmul(
                        ps_a_w, zaA1[:, i, :], x1[:, gi, :],
                        start=False, stop=(i == hb - 1),
                    )
                    nc.tensor.matmul(
                        pB, zaB1[:, i, :], x1[:, gj, :],
                        start=False, stop=(i == hb - 1),
                    )
                out_s = outp.tile([32 + hb, 256], f32, tag="outs")
                nc.vector.tensor_copy(out=out_s[0:hb, :], in_=ps_a_w)
                nc.scalar.copy(out=out_s[32 : 32 + hb, :], in_=pB)
                nc.gpsimd.dma_start(
                    out=out2[b0 : b0 + hb, :], in_=out_s[0:hb, :]
                )
                nc.gpsimd.dma_start(
                    out=out2[b0 + hb : b0 + BLK, :],
                    in_=out_s[32 : 32 + hb, :],
                )

            # ---- software-pipelined main loop ----
            # Loads: pairs (0,1)..(30,31), kept ~3 ahead of the consumer
            # (bufs=4 on x0/nfT).  lmask is queued behind the first pair.
            sm_prev = None
            z_next = build_z(0)
            nc.gpsimd.dma_start(out=lm_all, in_=lmask[:, :, :])
            load_x2(0)
            load_x2(2)
            for bb in range(NG):
                z_cur = z_next
                bb_, ps_a, x0, x1, goff, w_ready = alpha_phase(
                    bb, z_cur, sm_prev
                )
                if bb + 1 < NG:
                    z_next = build_z(bb + 1)
                if w_ready is not None:
                    weighted_phase(w_ready)
                if bb % 2 == 0 and bb + 4 <= 30:
                    load_x2(bb + 4)
                sm_prev = softmax_phase((bb_, ps_a, x0, x1, goff))
            weighted_phase(prep_weighted(sm_prev))

    nc.finalize()
    return nc


_NC_CACHE = None


def _get_nc():
    global _NC_CACHE
    if _NC_CACHE is None:
        _NC_CACHE = gen_kernel()
    return _NC_CACHE


def build_in_maps(target_feats, neighbor_feats, neighbor_label, hist_mask, W):
    target_feats = np.asarray(target_feats, dtype=np.float32)
    neighbor_feats = np.asarray(neighbor_feats, dtype=np.float32)
    neighbor_label = np.asarray(neighbor_label, dtype=np.float32)
    W = np.asarray(W, dtype=np.float32)

    # host-prepacked weight/target layouts matching the on-chip tiles
    wtr = np.ascontiguousarray(
        W.T.reshape(8, 128, D).transpose(1, 0, 2)
    ).astype(np.float16)                                      # [128, 8, D]
    lmask_full = np.where(np.asarray(hist_mask) > 0, 0.0, NEG).astype(np.float16)
    ident = np.eye(128, dtype=np.float16)
    m32 = np.zeros((128, BLK, BLK), dtype=np.float16)
    for i in range(BLK):
        m32[:, i, i] = 1.0

    in_maps = []
    for c in range(NCORES):
        s = slice(c * BC, (c + 1) * BC)
        # k-major interleaved nf|nl: xh[k, b, 0:128]=nf, xh[k, b, 128:256]=nl
        xh = np.empty((K, BC, 2 * D), dtype=np.float16)
        xh[:, :, 0:D] = neighbor_feats[s].transpose(1, 0, 2)
        xh[:, :, D : 2 * D] = neighbor_label[s].transpose(1, 0, 2)
        nfth = np.ascontiguousarray(
            neighbor_feats[s].transpose(2, 0, 1)
        ).astype(np.float16)
        tftr = np.ascontiguousarray(
            target_feats[s].T.reshape(8, 128, BC).transpose(1, 0, 2)
        ).astype(np.float16)                                  # [128, 8, BC]
        # lmask regrouped [BLK, NG, K]: partition = batch-within-group
        lm = np.ascontiguousarray(
            lmask_full[s].reshape(NG, BLK, K).transpose(1, 0, 2)
        )
        in_maps.append({
            "tftr": tftr,
            "wtr": wtr,
            "xh": xh,
            "nfth": nfth,
            "lmask": lm,
            "ident": ident,
            "m32h": m32,
        })
    return in_maps


def kernel(target_feats, neighbor_feats, neighbor_label, hist_mask, W):
    from concourse.bass_utils import run_bass_kernel_spmd

    in_maps = build_in_maps(
        target_feats, neighbor_feats, neighbor_label, hist_mask, W
    )
    nc = _get_nc()
    res = run_bass_kernel_spmd(nc, in_maps, list(range(NCORES))).results

    out = np.concatenate([res[c]["out2"] for c in range(NCORES)], axis=0)
    return np.ascontiguousarray(out[:, :D]), np.ascontiguousarray(out[:, D:])
